# revision 1
# baseline (speedup 1.0000x reference)
"""Trainium2 Bass kernel for nn_DenseTf: out = x @ sign(clip(w,-1,1)) + b.

Shapes (hardcoded from the problem spec):
    x: [8192, 4096] f32, w: [4096, 4096] f32, b: [4096] f32 -> out [8192, 4096] f32

Strategy: data-parallel over tokens across 8 NeuronCores. Each core computes
    out_c [1024, 4096] = x_c [1024, 4096] @ sign(w) [4096, 4096] + b
as an fp16 tensor-engine matmul with fp32 PSUM accumulation:
  - x_c loads f32 in natural layout (scalar HWDGE ring), is cast f32->fp16 on
    the DVE, and is transposed on the PE (identity matmuls of 128x128 blocks,
    PSUM->xT copy) into a resident SBUF tile xT [128, 32, 1024] (partition =
    d_in%128, mid = d_in//128 block, free = token). This avoids the gpsimd
    cast-DMA DRAM round trip of the earlier version (-16.8MB HBM per run,
    ~75us median win under shared-device bandwidth contention).
  - w streams in once as 1MB f32 quad tiles [128, 4, 512] on the sync ring;
    the DVE binarizes to {-0.5,+0.5} fp16 ((w>=0)-0.5; the *2 folds into the
    evict) into one of two persistent full-chunk wb buffers (wmode="dbuf").
  - w-prep for filter-chunk f+1 is software-pipelined under f's matmuls, so
    at an f-boundary the next wb chunk is already resident and neither the
    sync ring nor the in-order DVE queue stalls the PE (worth ~25% e2e).
  - matmuls: lhsT (stationary) = xT[:, k, m*128:(m+1)*128], rhs (moving) =
    wb tile [128, 512]; 8 PSUM banks hold the 8 token-tiles of one 512-wide
    filter chunk, accumulated over all 32 k-tiles (m-inner, k-outer).
  - evict fuses 2*psum+bias into one DVE scalar_tensor_tensor op; out-DMAs
    ride the scalar (ACT HWDGE) ring to keep the sync ring free for w.
  - bias is folded in via two K=1 matmuls against an fp16 hi/lo split of b
    (exact for b=0, ~fp32-accurate otherwise).
Timing note: no NTFF profiling exists in this container; device time is
measured by replication differencing (see test.py). Shared-device contention
makes single measurements noisy (+-15%); medians of 3 pairs are reported.
"""

import os

import numpy as np

N_CORES = 8
N_TOKENS = 8192
D_IN = 4096
FILTERS = 4096
P = 128

# Populated by kernel() after each run (BassKernelResults); test harness reads
# exec_time_ns off this.
LAST_RESULT = None

_CACHE = {}


def _build(m_per_core=N_TOKENS // N_CORES, d_in=D_IN, filters=FILTERS, fc=512, kq=4,
           reps=1, rep_xprep=True, rep_bias=True, mm_only=False, no_sign=False,
           ws_bufs=2, wb_bufs=12, out_bufs=4, binarize="dve", xprep="pe",
           waves=1, pipe=True, out_eng="scalar", fuse_evict=True, wmode="dbuf",
           w_prologue=True, xcopy_eng="scalar"):
    """Build + compile the single-core Bass program (SPMD across cores).

    reps>1 replicates the whole body inside one NEFF (timing only: wall-clock
    differencing against reps=1 cancels the axon dispatch overhead).
    rep_xprep/rep_bias control whether those phases replicate too (for
    bisection of where HW time goes)."""
    import concourse.mybir as mybir
    import concourse.tile as tile
    from concourse import bacc

    DT = mybir.dt.float16            # matmul dtype (fp16: 1 cyc/row, 10-bit mantissa)
    m_tiles = m_per_core // P        # token tiles of 128
    k_tiles = d_in // P              # contraction tiles of 128
    n_fc = filters // fc             # filter chunks
    n_kq = k_tiles // kq             # w DMA quads per chunk
    q_d = kq * P                     # d_in columns per x-prep chunk (matches kq)
    n_q = d_in // q_d

    nc = bacc.Bacc("TRN2", debug=False, target_bir_lowering=False)

    x_d = nc.dram_tensor("x", [m_per_core, d_in], mybir.dt.float32, kind="ExternalInput")
    w_d = nc.dram_tensor("w", [d_in, filters], mybir.dt.float32, kind="ExternalInput")
    b_d = nc.dram_tensor("b", [filters], mybir.dt.float32, kind="ExternalInput")
    o_d = nc.dram_tensor("out", [m_per_core, filters], mybir.dt.float32, kind="ExternalOutput")

    w_v = w_d[:].rearrange("(ko p) f -> p ko f", p=P)  # [128, k_tiles, filters]

    with tile.TileContext(nc) as tc:
        with (
            tc.tile_pool(name="dram", bufs=1, space="DRAM") as dram_pool,
            tc.tile_pool(name="xt", bufs=1) as xt_pool,
            tc.tile_pool(name="const", bufs=1) as const_pool,
            tc.tile_pool(name="bstage", bufs=2) as bs_pool,
            tc.tile_pool(name="wstage", bufs=ws_bufs) as ws_pool,
            tc.tile_pool(name="wbin", bufs=wb_bufs) as wb_pool,
            tc.tile_pool(name="outs", bufs=out_bufs) as out_pool,
        ):
            state = {}

            def emit_xprep_chunk(q):
                # x prep for one d_in chunk: gpsimd cast-DMA f32->fp16 into
                # DRAM scratch, then XBAR transpose-load (ACT HWDGE ring,
                # parallel to the SP ring carrying w/out).
                xbf, xT = state["xbf"], state["xT"]
                dsl = slice(q * q_d, (q + 1) * q_d)
                nc.gpsimd.dma_start(xbf[:, dsl], x_d[:, dsl])      # SWDGE cast
                nc.scalar.dma_start(
                    xT[:, q * kq:(q + 1) * kq, :], xbf[:, dsl], transpose=True
                )

            def alloc_x_tiles():
                if xprep != "pe":
                    state["xbf"] = dram_pool.tile([m_per_core, d_in], DT,
                                                  name="xbf")
                state["xT"] = xt_pool.tile([P, k_tiles, m_per_core], DT, name="xT")

            def emit_xprep_pe():
                # x prep without the DRAM round trip: load x f32 naturally
                # (scalar HWDGE ring), cast f32->fp16 on DVE, transpose
                # 128x128 blocks on the PE (identity matmul), copy PSUM->xT.
                # Saves 16.8MB/rep of HBM traffic vs the gpsimd-cast+XBAR
                # path; d_in-chunk-outer order matches the f=0 quad
                # consumption order so matmuls start after ~1 chunk.
                alloc_x_tiles()
                xT = state["xT"]
                # f0 w-prep first: its binarizes land in the DVE queue ahead
                # of the xprep casts, so f0 matmuls aren't stalled behind
                # ~40us of x-cast DVE work (scheduling only, no numeric
                # change; the sync ring is idle at rep start anyway).
                if w_prologue and pipe and wmode == "dbuf" and not (
                        mm_only or no_sign):
                    state["wbA"] = xt_pool.tile([P, k_tiles, fc], DT,
                                                name="wbA")
                    state["wbB"] = xt_pool.tile([P, k_tiles, fc], DT,
                                                name="wbB")
                    for qi in range(n_kq):
                        emit_wprep_into(state["wbA"], 0, qi, None)
                from concourse import masks
                if "idn" not in state:
                    idn = const_pool.tile([P, P], DT, name="idn")
                    masks.make_identity(nc, idn[:])
                    state["idn"] = idn
                idn = state["idn"]
                cd = 1024            # d_in columns per chunk (8 k-tiles)
                n_c = d_in // cd
                kpc = cd // P
                with (
                    tc.tile_pool(name="xs", bufs=2) as xs_pool,
                    tc.tile_pool(name="xs16", bufs=2) as x16_pool,
                    tc.tile_pool(name="psum_t", bufs=2, space="PSUM") as pt_pool,
                ):
                    for c in range(n_c):
                        for m in range(m_tiles):
                            xs = xs_pool.tile([P, cd], mybir.dt.float32,
                                              tag="xs", name="xs")
                            nc.scalar.dma_start(
                                xs[:], x_d[m * P:(m + 1) * P,
                                           c * cd:(c + 1) * cd])
                            x16 = x16_pool.tile([P, cd], DT, tag="x16",
                                                name="x16")
                            nc.vector.tensor_copy(x16[:], xs[:])
                            pt = pt_pool.tile([P, kpc, P], DT, tag="pt",
                                              name="pt")
                            for kk in range(kpc):
                                nc.tensor.matmul(
                                    pt[:, kk, :], x16[:, kk * P:(kk + 1) * P],
                                    idn[:], is_transpose=True)
                            xdst = xT[:, c * kpc:(c + 1) * kpc,
                                      m * P:(m + 1) * P]
                            if xcopy_eng == "scalar":
                                nc.scalar.copy(xdst, pt[:])
                            else:
                                nc.vector.tensor_copy(xdst, pt[:])

            def emit_xprep():
                alloc_x_tiles()
                xbf, xT = state["xbf"], state["xT"]
                for q in range(n_q):
                    dsl = slice(q * q_d, (q + 1) * q_d)
                    nc.gpsimd.dma_start(xbf[:, dsl], x_d[:, dsl])  # SWDGE cast
                for q in range(n_q):
                    dsl = slice(q * q_d, (q + 1) * q_d)
                    nc.scalar.dma_start(
                        xT[:, q * kq:(q + 1) * kq, :], xbf[:, dsl], transpose=True
                    )

            def emit_bias():
                # bias: hi/lo fp16 split, broadcast to [128, filters] via PE
                # (ones[1,128].T @ b[1,:]); runs inside the startup bubble.
                ones_sb = const_pool.tile([1, P], DT, name="ones_sb")
                nc.any.memset(ones_sb[:], 1.0)
                b_hi = const_pool.tile([1, filters], DT, name="b_hi")
                b_lo = const_pool.tile([1, filters], DT, name="b_lo")
                bias_bc = const_pool.tile([P, filters], mybir.dt.float32,
                                          name="bias_bc")
                for i in range(n_fc):
                    sl = slice(i * fc, (i + 1) * fc)
                    bs = bs_pool.tile([1, fc], mybir.dt.float32, tag="bs", name="bs")
                    nc.sync.dma_start(bs[:], b_d[None, sl])
                    nc.vector.tensor_copy(b_hi[:, sl], bs[:])     # hi = fp16(b)
                    bh32 = bs_pool.tile([1, fc], mybir.dt.float32, tag="bh32",
                                        name="bh32")
                    nc.vector.tensor_copy(bh32[:], b_hi[:, sl])
                    nc.vector.tensor_sub(bs[:], bs[:], bh32[:])   # residual
                    nc.vector.tensor_copy(b_lo[:, sl], bs[:])     # lo = fp16(b-hi)
                with tc.tile_pool(name="psum_b", bufs=n_fc, space="PSUM") as psum_b:
                    for i in range(n_fc):
                        sl = slice(i * fc, (i + 1) * fc)
                        pb = psum_b.tile([P, fc], mybir.dt.float32, tag="pb",
                                         name="pb")
                        nc.tensor.matmul(pb[:], ones_sb[:1, :], b_hi[:1, sl],
                                         start=True, stop=False)
                        nc.tensor.matmul(pb[:], ones_sb[:1, :], b_lo[:1, sl],
                                         start=False, stop=True)
                        nc.vector.tensor_copy(bias_bc[:, sl], pb[:])
                state["bias_bc"] = bias_bc

            def emit_wprep(f, qi, wb_const):
                fsl = slice(f * fc, (f + 1) * fc)
                if mm_only:
                    return wb_const
                ws = ws_pool.tile([P, kq, fc], mybir.dt.float32,
                                  tag="ws", name="ws")
                nc.sync.dma_start(ws[:], w_v[:, qi * kq:(qi + 1) * kq, fsl])
                if no_sign:
                    # diagnostic: DMA w but matmul a const tile
                    nc.vector.tensor_copy(ws[:1, :1, :8], ws[:1, :1, :8])
                    return wb_const
                wb = wb_pool.tile([P, kq, fc], DT, tag="wb", name="wb")
                if binarize == "dve":
                    # (w >= 0) - 0.5 -> {-0.5, +0.5}; the *2 folds into evict
                    nc.vector.tensor_scalar(
                        wb[:], ws[:], 0.0, 0.5,
                        mybir.AluOpType.is_ge, mybir.AluOpType.subtract)
                else:
                    nc.scalar.sign(wb[:], ws[:])                  # binarize+cast
                return wb

            def emit_evict(f, m, psum_m):
                fsl = slice(f * fc, (f + 1) * fc)
                bias_bc = state["bias_bc"]
                ot = out_pool.tile([P, fc], mybir.dt.float32, tag="ot", name="ot")
                if binarize == "dve" and not (mm_only or no_sign):
                    # weights were {+-0.5}: out = 2*psum + bias
                    if fuse_evict:
                        nc.vector.scalar_tensor_tensor(
                            ot[:], psum_m[:], 2.0, bias_bc[:, fsl],
                            mybir.AluOpType.mult, mybir.AluOpType.add)
                    else:
                        nc.vector.tensor_scalar(ot[:], psum_m[:], 2.0, None,
                                                mybir.AluOpType.mult)
                        nc.vector.tensor_add(ot[:], ot[:], bias_bc[:, fsl])
                else:
                    nc.vector.tensor_add(ot[:], psum_m[:], bias_bc[:, fsl])
                o_eng = nc.scalar if out_eng == "scalar" else nc.sync
                o_eng.dma_start(o_d[m * P:(m + 1) * P, fsl], ot[:])

            def emit_mm_group(f, qi, wb, psums, wave_ms):
                xT = state["xT"]
                for kk in range(kq):
                    k = qi * kq + kk
                    for m in wave_ms:
                        nc.tensor.matmul(
                            psums[m][:],
                            xT[:, k, m * P:(m + 1) * P],
                            wb[:, kk, :],
                            start=(k == 0),
                            stop=(k == k_tiles - 1),
                        )

            def emit_main():
                # main loop: stream w once, binarize, matmul
                wb_const = None
                if mm_only or no_sign:
                    wb_const = const_pool.tile([P, kq, fc], DT, name="wb_const")
                    nc.any.memset(wb_const[:], 1.0)
                if pipe:
                    emit_main_pipe(wb_const)
                    return
                mw = m_tiles // waves
                with tc.tile_pool(name="psum", bufs=m_tiles, space="PSUM") as pp:
                    for f in range(n_fc):
                        psums = {}
                        wbs = {}
                        for wv in range(waves):
                            wave_ms = range(wv * mw, (wv + 1) * mw)
                            for m in wave_ms:
                                psums[m] = pp.tile([P, fc], mybir.dt.float32,
                                                   tag="ps", name=f"ps_{f}_{m}")
                            for qi in range(n_kq):
                                if xprep == "interleave" and f == 0 and wv == 0:
                                    emit_xprep_chunk(qi)
                                if wv == 0:
                                    wbs[qi] = emit_wprep(f, qi, wb_const)
                                emit_mm_group(f, qi, wbs[qi], psums, wave_ms)
                            for m in wave_ms:
                                emit_evict(f, m, psums[m])

            def emit_wprep_into(dst, f, qi, wb_const):
                # w-prep writing into a slice of a persistent per-chunk wb
                # buffer (wmode="dbuf"): DMA quad -> staging, binarize into
                # dst[:, qi*kq:(qi+1)*kq, :].
                fsl = slice(f * fc, (f + 1) * fc)
                if mm_only:
                    return
                ws = ws_pool.tile([P, kq, fc], mybir.dt.float32,
                                  tag="ws", name="ws")
                nc.sync.dma_start(ws[:], w_v[:, qi * kq:(qi + 1) * kq, fsl])
                if no_sign:
                    nc.vector.tensor_copy(ws[:1, :1, :8], ws[:1, :1, :8])
                    return
                ksl = slice(qi * kq, (qi + 1) * kq)
                nc.vector.tensor_scalar(
                    dst[:, ksl, :], ws[:], 0.0, 0.5,
                    mybir.AluOpType.is_ge, mybir.AluOpType.subtract)

            def emit_main_pipe(wb_const):
                # Software-pipelined main loop: w-prep (DMA + binarize) for
                # f-chunk f+1 is interleaved with f's matmul groups, so at an
                # f-boundary the next chunk's wb tiles are already in SBUF and
                # neither the sync ring (w DMAs) nor the DVE queue (binarize)
                # sits behind f's evict work. Out-DMAs ride the scalar ring.
                dbuf = wmode == "dbuf" and not (mm_only or no_sign)
                f0_prefilled = False
                if dbuf:
                    if "wbA" in state:
                        wbA = state.pop("wbA")
                        wbB = state.pop("wbB")
                        f0_prefilled = True
                    else:
                        wbA = xt_pool.tile([P, k_tiles, fc], DT, name="wbA")
                        wbB = xt_pool.tile([P, k_tiles, fc], DT, name="wbB")

                    def wb_of(f):
                        return wbA if f % 2 == 0 else wbB

                wbs = {}
                with tc.tile_pool(name="psum", bufs=m_tiles, space="PSUM") as pp:
                    for f in range(n_fc):
                        psums = {}
                        for m in range(m_tiles):
                            psums[m] = pp.tile([P, fc], mybir.dt.float32,
                                               tag="ps", name=f"ps_{f}_{m}")
                        for qi in range(n_kq):
                            if f == 0:
                                if xprep == "interleave":
                                    emit_xprep_chunk(qi)
                                if dbuf:
                                    if not f0_prefilled:
                                        emit_wprep_into(wb_of(0), 0, qi,
                                                        wb_const)
                                else:
                                    wbs[(0, qi)] = emit_wprep(0, qi, wb_const)
                            if dbuf:
                                xT = state["xT"]
                                wbf = wb_of(f)
                                for kk in range(kq):
                                    k = qi * kq + kk
                                    for m in range(m_tiles):
                                        nc.tensor.matmul(
                                            psums[m][:],
                                            xT[:, k, m * P:(m + 1) * P],
                                            wbf[:, k, :],
                                            start=(k == 0),
                                            stop=(k == k_tiles - 1),
                                        )
                            else:
                                emit_mm_group(f, qi, wbs.pop((f, qi)), psums,
                                              range(m_tiles))
                            if f + 1 < n_fc:
                                if dbuf:
                                    emit_wprep_into(wb_of(f + 1), f + 1, qi,
                                                    wb_const)
                                else:
                                    wbs[(f + 1, qi)] = emit_wprep(f + 1, qi,
                                                                  wb_const)
                        for m in range(m_tiles):
                            emit_evict(f, m, psums[m])

            def do_xprep():
                if xprep == "interleave":
                    alloc_x_tiles()
                elif xprep == "pe":
                    emit_xprep_pe()
                else:
                    emit_xprep()

            if not rep_xprep:
                do_xprep()
            if not rep_bias:
                emit_bias()
            for _rep in range(reps):
                if rep_xprep:
                    do_xprep()
                if rep_bias:
                    emit_bias()
                emit_main()

    nc.compile()
    return nc


def _get_nc():
    key = "full"
    if key not in _CACHE:
        _CACHE[key] = _build()
    return _CACHE[key]


_RUNNER = {}


def _get_runner():
    """Jitted 8-core shard_map callable around the compiled NEFF, cached so
    repeat kernel() calls skip retracing. x shards over tokens (axis 0); w and
    b replicate via PartitionSpec() (no host-side 8x concat)."""
    if "fn" in _RUNNER:
        return _RUNNER["fn"]
    import jax
    from jax.sharding import Mesh, PartitionSpec
    from jax.experimental.shard_map import shard_map
    from concourse import bass2jax, mybir

    nc_mod = _get_nc()
    bass2jax.install_neuronx_cc_hook()
    partition_name = (
        nc_mod.partition_id_tensor.name if nc_mod.partition_id_tensor else None
    )
    in_names, out_names, out_avals, zero_shapes = [], [], [], []
    for alloc in nc_mod.m.functions[0].allocations:
        if not isinstance(alloc, mybir.MemoryLocationSet):
            continue
        name = alloc.memorylocations[0].name
        if alloc.kind == "ExternalInput":
            if name != partition_name:
                in_names.append(name)
        elif alloc.kind == "ExternalOutput":
            shape = tuple(alloc.tensor_shape)
            dtype = mybir.dt.np(alloc.dtype)
            out_names.append(name)
            out_avals.append(jax.core.ShapedArray(shape, dtype))
            zero_shapes.append((shape, dtype))

    def _body(*args):
        operands = list(args)
        if partition_name is not None:
            operands.append(bass2jax.partition_id_tensor())
        outs = bass2jax._bass_exec_p.bind(
            *operands,
            out_avals=tuple(out_avals),
            in_names=tuple(
                in_names + out_names
                + ([partition_name] if partition_name else [])
            ),
            out_names=tuple(out_names),
            lowering_input_output_aliases=(),
            sim_require_finite=True,
            sim_require_nnan=True,
            nc=nc_mod,
        )
        return tuple(outs)

    devices = jax.devices()[:N_CORES]
    mesh = Mesh(np.asarray(devices), ("core",))
    spec_of = {"x": PartitionSpec("core"), "w": PartitionSpec(),
               "b": PartitionSpec()}
    in_specs = tuple(spec_of[n] for n in in_names) + (
        PartitionSpec("core"),
    ) * len(out_names)
    out_specs = (PartitionSpec("core"),) * len(out_names)
    fn = jax.jit(
        shard_map(_body, mesh=mesh, in_specs=in_specs, out_specs=out_specs,
                  check_rep=False),
        keep_unused=True,
    )
    _RUNNER["fn"] = (fn, in_names, zero_shapes)
    return _RUNNER["fn"]


def kernel(x, w, b):
    x = np.ascontiguousarray(np.asarray(x, dtype=np.float32))
    w = np.ascontiguousarray(np.asarray(w, dtype=np.float32))
    b = np.ascontiguousarray(np.asarray(b, dtype=np.float32))

    fn, in_names, zero_shapes = _get_runner()
    arrs = {"x": x, "w": w, "b": b}
    zeros = [
        np.zeros((N_CORES * shape[0], *shape[1:]), dtype)
        for shape, dtype in zero_shapes
    ]
    outs = fn(*[arrs[n] for n in in_names], *zeros)
    return np.asarray(outs[0])

